# revision 11
# baseline (speedup 1.0000x reference)
"""Binary associative memory (causal linear attention with binarized k/v).

Self-contained Trainium2 Bass kernel.

Math: the reference's chunked prefix recurrence telescopes to exact causal
linear attention:
    out[t] = (1/(8*(t+1))) * sum_{s<=t} (q[t].k[s]) v[s],   k,v = sign(qkv)
    y      = out @ W_o.T   (summed over head features)
    final_matrix[b,h] = sum_t k[t] (x) v[t]   (exact integers)
so we are free to re-chunk at 128 tokens (partition width).

Sharding: 8 cores = 4 batches x 2 head-groups (8 heads each).

Precision: k/v binarize must match the fp32 reference's signs, so the k/v
projections use an fp16 double-double trick: x ~= x1 + x2 and W ~= w1 + w2
(fp16 splits, prepared on host) stacked along the contraction dim, so one
K=2048 accumulation computes (x1+x2)@(w1+w2) with ~1e-7 relative error at
full PE speed. q (continuous) uses a single fp16 term; o_proj runs fp16.

Layouts per core:
  - q,k projected feature-on-partition ([feat, tok]) for scoresT/crossT
  - v projected token-on-partition directly ([tok, vfeat]) for intra/ckv
  - k transposed per chunk on the PE (with identity) for ckv
  - state S[kf, df] fp16 (exact small integers), 2 heads per 128 partitions
"""

import functools

import numpy as np

T = 4096
D = 1024
HLOC = 8  # heads per core
DH = 64
CH = 128  # chunk size
NCH = T // CH  # 32
TT = 512  # projection token tile
NTT = T // TT  # 8

PAIRS = [(0, 2), (1, 3), (4, 6), (5, 7)]  # scores PSUM-tile pairs (same row-group)
# 3-term fp16 split of x@W: x1w1 + x2w1 + x1w2 (chunk indices into [hi(8), lo(8)])
SPLIT3 = [(wc, xc) for wc, xc in
          [(c, c) for c in range(8)] + [(c, c + 8) for c in range(8)] + [(c + 8, c) for c in range(8)]]


@functools.lru_cache(maxsize=1)
def _build():
    from contextlib import ExitStack

    import concourse.bacc as bacc
    import concourse.mybir as mybir
    import concourse.tile as tile

    f32 = mybir.dt.float32
    f16 = mybir.dt.float16

    nc = bacc.Bacc("TRN2", target_bir_lowering=False, debug=False, num_devices=8)

    # x split [x1; x2], chunk-major: [16 kc][128][T]
    xs = nc.dram_tensor("xs", [16, 128, T], f16, kind="ExternalInput").ap()
    wq = nc.dram_tensor("wq", [8, 128, 512], f16, kind="ExternalInput").ap()
    wk = nc.dram_tensor("wk", [16, 128, 512], f16, kind="ExternalInput").ap()
    wv = nc.dram_tensor("wv", [16, 128, 512], f16, kind="ExternalInput").ap()
    wo = nc.dram_tensor("wo", [4, 128, 1024], f16, kind="ExternalInput").ap()
    invtot = nc.dram_tensor("invtot", [128, T], f32, kind="ExternalInput").ap()
    mask2 = nc.dram_tensor("mask2", [128, 256], f32, kind="ExternalInput").ap()
    ident = nc.dram_tensor("ident", [128, 128], f16, kind="ExternalInput").ap()
    ypart = nc.dram_tensor("ypart", [T, D], f32, kind="ExternalOutput").ap()
    fmat = nc.dram_tensor("fmat", [HLOC, DH, DH], f32, kind="ExternalOutput").ap()

    with tile.TileContext(nc) as tc, ExitStack() as ctx:
        const = ctx.enter_context(tc.tile_pool(name="const", bufs=1))
        xpool = ctx.enter_context(tc.tile_pool(name="xp", bufs=2))
        qkpool = ctx.enter_context(tc.tile_pool(name="qk", bufs=2))
        tokpool = ctx.enter_context(tc.tile_pool(name="tok", bufs=3))
        stpool = ctx.enter_context(tc.tile_pool(name="st", bufs=8))
        opool = ctx.enter_context(tc.tile_pool(name="op", bufs=3))
        ypool = ctx.enter_context(tc.tile_pool(name="yp", bufs=3))
        spool = ctx.enter_context(tc.tile_pool(name="sp", bufs=2))
        pspool = ctx.enter_context(tc.tile_pool(name="ps", bufs=8, space="PSUM"))

        # weights needed by the first projections: emit DMAs first
        wq_sb = const.tile([128, 8, 512], f16)
        for kc in range(8):
            nc.sync.dma_start(wq_sb[:, kc, :], wq[kc])
        wk_sb = const.tile([128, 16, 512], f16)
        for kc in range(16):
            nc.sync.dma_start(wk_sb[:, kc, :], wk[kc])

        xs_tiles = {}

        def dma_x(tt, kc_lo, kc_hi):
            if tt >= NTT:
                return
            if tt not in xs_tiles:
                xs_tiles[tt] = xpool.tile([128, 16, TT], f16, tag="xs", name="xs_sb")
            t0 = tt * TT
            for kc in range(kc_lo, kc_hi):
                nc.sync.dma_start(xs_tiles[tt][:, kc, :], xs[kc, :, t0 : t0 + TT])

        qk_tiles = {}

        def proj_qk(tt, fb):
            """Emit projection of q and k feature-block fb (0..3) of tile tt."""
            if tt >= NTT:
                return
            if tt not in qk_tiles:
                qk_tiles[tt] = (
                    qkpool.tile([128, 4 * TT], f16, tag="qT", name="qT_sb"),
                    qkpool.tile([128, 4 * TT], f16, tag="kT", name="kT_sb"),
                )
            qT, kT = qk_tiles[tt]
            xt = xs_tiles[tt]
            pq = pspool.tile([128, 512], f32, tag="work", name="pq")
            for kc in range(8):
                nc.tensor.matmul(
                    pq,
                    wq_sb[:, kc, fb * 128 : (fb + 1) * 128],
                    xt[:, kc, :],
                    start=(kc == 0),
                    stop=(kc == 7),
                )
            nc.scalar.copy(qT[:, fb * TT : (fb + 1) * TT], pq)
            pk = pspool.tile([128, 512], f32, tag="work", name="pk")
            for i, (wc, xc) in enumerate(SPLIT3):
                nc.tensor.matmul(
                    pk,
                    wk_sb[:, wc, fb * 128 : (fb + 1) * 128],
                    xt[:, xc, :],
                    start=(i == 0),
                    stop=(i == len(SPLIT3) - 1),
                )
            nc.scalar.sign(kT[:, fb * TT : (fb + 1) * TT], pk)

        # prologue: x(0) + full projection of tile 0
        dma_x(0, 0, 16)
        for fb in range(4):
            proj_qk(0, fb)

        # remaining constants (deferred so they queue behind the hot DMAs)
        wv_sb = const.tile([128, 16, 512], f16)
        for kc in range(16):
            nc.sync.dma_start(wv_sb[:, kc, :], wv[kc])
        wo_sb = const.tile([128, 4, 1024], f16)
        nc.sync.dma_start(wo_sb, wo.rearrange("jc p i -> p jc i"))
        inv_sb = const.tile([128, T], f32)
        nc.sync.dma_start(inv_sb, invtot)
        mask_sb = const.tile([128, 256], f32)
        nc.sync.dma_start(mask_sb, mask2)
        id_sb = const.tile([128, 128], f16)
        nc.sync.dma_start(id_sb, ident)

        # running state S[kf, df] fp16 (exact ints), 2 heads stacked per tile
        s_cur = spool.tile([128, 256], f16, tag="S", name="s_init")
        nc.vector.memset(s_cur, 0.0)

        for tt in range(NTT):
            qT, kT = qk_tiles[tt]
            xt = xs_tiles[tt]
            for ci in range(4):
                n = tt * 4 + ci
                c0 = ci * CH

                # interleave next tile's x DMA + projection to keep PE dense
                if ci == 0:
                    dma_x(tt + 1, 0, 16)
                if tt + 1 < NTT:
                    proj_qk(tt + 1, ci)

                # --- v projection straight to token-partition [t, vfeat] ---
                pv = pspool.tile([128, 512], f32, tag="work", name="pv")
                for i, (wc, xc) in enumerate(SPLIT3):
                    nc.tensor.matmul(
                        pv,
                        xt[:, xc, c0 : c0 + CH],
                        wv_sb[:, wc, :],
                        start=(i == 0),
                        stop=(i == len(SPLIT3) - 1),
                    )
                vtok = tokpool.tile([128, 512], f16, tag="vtok")
                nc.scalar.sign(vtok, pv)

                # --- scoresT[j, i], masked -> fp16. Heads sharing a PSUM
                # tile use the same PE row-group (concurrent different-row
                # matmuls on one bank collide fatally). ---
                scs = [
                    pspool.tile([128, 256], f32, tag="work", name=f"sc{p}")
                    for p in range(4)
                ]
                for h in range(8):  # emission order alternates row-groups
                    p = (h % 2) + 2 * (h // 4)
                    idx = (h // 2) % 2
                    bp = (h % 2) * 64
                    hp = h // 2
                    sl = slice(hp * TT + c0, hp * TT + c0 + CH)
                    nc.tensor.matmul(
                        scs[p][:, idx * 128 : (idx + 1) * 128],
                        kT[bp : bp + 64, sl],
                        qT[bp : bp + 64, sl],
                        start=True,
                        stop=True,
                    )
                sts = []
                for p in range(4):
                    st = stpool.tile([128, 256], f16, tag="st")
                    nc.vector.tensor_mul(st, scs[p], mask_sb)
                    sts.append(st)

                # --- transpose k head-pair blocks -> token-partition ---
                ktp = pspool.tile([128, 512], f16, tag="work", name="ktp")
                for hp in range(4):
                    sl = slice(hp * TT + c0, hp * TT + c0 + CH)
                    nc.tensor.transpose(
                        ktp[:, hp * 128 : (hp + 1) * 128], kT[:, sl], id_sb
                    )
                ktok = tokpool.tile([128, 512], f16, tag="ktok")
                nc.vector.tensor_copy(ktok, ktp)

                # --- ckv[kf, df] per head ---
                ckv = pspool.tile([128, 256], f32, tag="work", name="ckv")
                for h in range(8):
                    bp = (h % 2) * 64
                    nc.tensor.matmul(
                        ckv[bp : bp + 64, (h // 2) * 64 : (h // 2) * 64 + 64],
                        ktok[:, h * 64 : h * 64 + 64],
                        vtok[:, h * 64 : h * 64 + 64],
                        start=True,
                        stop=True,
                        tile_position=(0, bp),
                    )

                # --- outT[dd, i] = intraT + crossT ---
                ot = pspool.tile([128, 512], f32, tag="work", name="ot")
                for h in range(8):
                    bp = (h % 2) * 64
                    hp = h // 2
                    sp_ = (h % 2) + 2 * (h // 4)
                    si_ = (h // 2) % 2
                    nc.tensor.matmul(
                        ot[bp : bp + 64, hp * 128 : hp * 128 + 128],
                        vtok[:, h * 64 : h * 64 + 64],
                        sts[sp_][:, si_ * 128 : si_ * 128 + 128],
                        start=True,
                        stop=(n == 0),
                        tile_position=(0, bp),
                    )
                    if n > 0:
                        nc.tensor.matmul(
                            ot[bp : bp + 64, hp * 128 : hp * 128 + 128],
                            s_cur[bp : bp + 64, hp * 64 : hp * 64 + 64],
                            qT[bp : bp + 64, hp * TT + c0 : hp * TT + c0 + CH],
                            start=False,
                            stop=True,
                            tile_position=(bp, bp),
                        )

                # --- state update S += ckv (exact ints in fp16) ---
                s_new = spool.tile([128, 256], f16, tag="S", name="s_new")
                nc.vector.tensor_add(s_new, s_cur, ckv)
                s_cur = s_new

                # --- scale by 1/(8*(t+1)), cast to fp16 for o_proj ---
                osb = opool.tile([128, 512], f16, tag="osb")
                for hp in range(4):
                    nc.vector.tensor_mul(
                        osb[:, hp * 128 : (hp + 1) * 128],
                        ot[:, hp * 128 : (hp + 1) * 128],
                        inv_sb[:, n * CH : n * CH + CH],
                    )

                # --- o_proj (fp16): y[t, i] partial over this core's j-feats ---
                ysb = ypool.tile([128, 1024], f32, tag="ysb")
                for icol in range(2):
                    yp = pspool.tile([128, 512], f32, tag="work", name="yp")
                    for hp in range(4):
                        nc.tensor.matmul(
                            yp,
                            osb[:, hp * 128 : (hp + 1) * 128],
                            wo_sb[:, hp, icol * 512 : (icol + 1) * 512],
                            start=(hp == 0),
                            stop=(hp == 3),
                        )
                    nc.scalar.copy(ysb[:, icol * 512 : (icol + 1) * 512], yp)
                nc.sync.dma_start(ypart[n * CH : (n + 1) * CH, :], ysb)

        # --- final matrix (exact integer sums) ---
        fsb = opool.tile([128, 256], f32, tag="fsb")
        nc.vector.tensor_copy(fsb, s_cur)
        for h in range(8):
            nc.sync.dma_start(
                fmat[h],
                fsb[(h % 2) * 64 : (h % 2) * 64 + 64, (h // 2) * 64 : (h // 2) * 64 + 64],
            )

    nc.compile()
    return nc


def _split16(a):
    """fp16 double-double split along axis 0-stacking: returns (hi, lo)."""
    hi = a.astype(np.float16)
    lo = (a - hi.astype(np.float32)).astype(np.float16)
    return hi, lo


def _host_inputs(x, W_qkv, W_o):
    f32 = np.float32
    tvec = np.arange(1, T + 1, dtype=np.float64)
    inv = (1.0 / (8.0 * tvec)).astype(f32)
    invtot = np.ascontiguousarray(np.broadcast_to(inv[None, :], (128, T)))
    jj = np.arange(128)
    maskT = (jj[:, None] <= jj[None, :]).astype(f32)
    mask2 = np.ascontiguousarray(np.tile(maskT, (1, 2)))
    ident = np.eye(128, dtype=np.float16)

    Wq3 = np.asarray(W_qkv, dtype=f32).reshape(3, 16, DH, D)
    in_maps = []
    for core in range(8):
        b, g = core // 2, core % 2
        xT = np.asarray(x[b], dtype=f32).T  # [D, T]
        x1, x2 = _split16(xT)
        xs = np.ascontiguousarray(
            np.concatenate([x1, x2], axis=0).reshape(16, 128, T)
        )
        # weight column blocks for this head group, [D, 512] each
        wqc = Wq3[0, g * 8 : (g + 1) * 8].reshape(512, D).T
        wkc = Wq3[1, g * 8 : (g + 1) * 8].reshape(512, D).T
        wvc = Wq3[2, g * 8 : (g + 1) * 8].reshape(512, D).T
        wq = np.ascontiguousarray(wqc.astype(np.float16).reshape(8, 128, 512))
        wk1, wk2 = _split16(wkc)
        wk = np.ascontiguousarray(np.concatenate([wk1, wk2], axis=0).reshape(16, 128, 512))
        wv1, wv2 = _split16(wvc)
        wv = np.ascontiguousarray(np.concatenate([wv1, wv2], axis=0).reshape(16, 128, 512))
        wo = np.ascontiguousarray(
            np.asarray(W_o, dtype=f32)[:, g * 512 : (g + 1) * 512].T.astype(np.float16).reshape(4, 128, D)
        )
        in_maps.append(
            {
                "xs": xs,
                "wq": wq,
                "wk": wk,
                "wv": wv,
                "wo": wo,
                "invtot": invtot,
                "mask2": mask2,
                "ident": ident,
            }
        )
    return in_maps


def kernel(x, W_qkv, W_o, trace=False):
    from concourse import bass_utils

    nc = _build()
    in_maps = _host_inputs(x, W_qkv, W_o)
    res = bass_utils.run_bass_kernel_spmd(
        nc, in_maps, core_ids=list(range(8)), trace=trace
    )
    results = res.results

    f32 = np.float32
    y = np.empty((4, T, D), dtype=f32)
    fm = np.empty((4, 16, DH, DH), dtype=f32)
    for core in range(8):
        b, g = core // 2, core % 2
        if g == 0:
            y[b] = results[core]["ypart"]
        else:
            y[b] += results[core]["ypart"]
        fm[b, g * 8 : (g + 1) * 8] = results[core]["fmat"]
    fc = np.full((4, 16, 1, 1), float(T), dtype=f32)
    if trace:
        kernel._last_result = res
    return (y, fm, fc)


# revision 14
# speedup vs baseline: 1.0606x; 1.0606x over previous
"""Binary associative memory (causal linear attention with binarized k/v).

Self-contained Trainium2 Bass kernel.

Math: the reference's chunked prefix recurrence telescopes to exact causal
linear attention:
    out[t] = (1/(8*(t+1))) * sum_{s<=t} (q[t].k[s]) v[s],   k,v = sign(qkv)
    y      = out @ W_o.T   (summed over head features)
    final_matrix[b,h] = sum_t k[t] (x) v[t]   (exact integers)
so we are free to re-chunk at 128 tokens (partition width).

Sharding: 8 cores = 4 batches x 2 head-groups (8 heads each).

Precision: k/v binarize must match the fp32 reference's signs, so the k/v
projections use an fp16 double-double trick: x ~= x1 + x2 and W ~= w1 + w2
(fp16 splits, prepared on host) stacked along the contraction dim, so one
K=2048 accumulation computes (x1+x2)@(w1+w2) with ~1e-7 relative error at
full PE speed. q (continuous) uses a single fp16 term; o_proj runs fp16.

Layouts per core:
  - q,k projected feature-on-partition ([feat, tok]) for scoresT/crossT
  - v projected token-on-partition directly ([tok, vfeat]) for intra/ckv
  - k transposed per chunk on the PE (with identity) for ckv
  - state S[kf, df] fp16 (exact small integers), 2 heads per 128 partitions
"""

import functools

import numpy as np

T = 4096
D = 1024
HLOC = 8  # heads per core
DH = 64
CH = 128  # chunk size
NCH = T // CH  # 32
TT = 512  # projection token tile
NTT = T // TT  # 8

PAIRS = [(0, 2), (1, 3), (4, 6), (5, 7)]  # scores PSUM-tile pairs (same row-group)
# 3-term fp16 split of x@W: x1w1 + x2w1 + x1w2 (chunk indices into [hi(8), lo(8)])
SPLIT3 = [(wc, xc) for wc, xc in
          [(c, c) for c in range(8)] + [(c, c + 8) for c in range(8)] + [(c + 8, c) for c in range(8)]]


@functools.lru_cache(maxsize=1)
def _build():
    from contextlib import ExitStack

    import concourse.bacc as bacc
    import concourse.mybir as mybir
    import concourse.tile as tile

    f32 = mybir.dt.float32
    f16 = mybir.dt.float16

    nc = bacc.Bacc("TRN2", target_bir_lowering=False, debug=False, num_devices=8)

    # x split [x1; x2], chunk-major: [16 kc][128][T]
    xs = nc.dram_tensor("xs", [16, 128, T], f16, kind="ExternalInput").ap()
    wq = nc.dram_tensor("wq", [8, 128, 512], f16, kind="ExternalInput").ap()
    wk = nc.dram_tensor("wk", [16, 128, 512], f16, kind="ExternalInput").ap()
    wv = nc.dram_tensor("wv", [16, 128, 512], f16, kind="ExternalInput").ap()
    wo = nc.dram_tensor("wo", [4, 128, 1024], f16, kind="ExternalInput").ap()
    invtot = nc.dram_tensor("invtot", [128, T], f32, kind="ExternalInput").ap()
    mask2 = nc.dram_tensor("mask2", [128, 256], f32, kind="ExternalInput").ap()
    ident = nc.dram_tensor("ident", [128, 128], f16, kind="ExternalInput").ap()
    ypart = nc.dram_tensor("ypart", [T, D], f32, kind="ExternalOutput").ap()
    fmat = nc.dram_tensor("fmat", [HLOC, DH, DH], f32, kind="ExternalOutput").ap()

    with tile.TileContext(nc) as tc, ExitStack() as ctx:
        const = ctx.enter_context(tc.tile_pool(name="const", bufs=1))
        xpool = ctx.enter_context(tc.tile_pool(name="xp", bufs=3))
        qkpool = ctx.enter_context(tc.tile_pool(name="qk", bufs=2))
        tokpool = ctx.enter_context(tc.tile_pool(name="tok", bufs=3))
        stpool = ctx.enter_context(tc.tile_pool(name="st", bufs=8))
        opool = ctx.enter_context(tc.tile_pool(name="op", bufs=3))
        ypool = ctx.enter_context(tc.tile_pool(name="yp", bufs=3))
        spool = ctx.enter_context(tc.tile_pool(name="sp", bufs=2))
        pspool = ctx.enter_context(tc.tile_pool(name="ps", bufs=8, space="PSUM"))

        # weights needed by the first projections: emit DMAs first
        wq_sb = const.tile([128, 8, 512], f16)
        for kc in range(8):
            nc.sync.dma_start(wq_sb[:, kc, :], wq[kc])
        wk_sb = const.tile([128, 16, 512], f16)
        for kc in range(16):
            nc.sync.dma_start(wk_sb[:, kc, :], wk[kc])

        xs_tiles = {}

        def dma_x(tt, kc_lo, kc_hi):
            if tt >= NTT:
                return
            if tt not in xs_tiles:
                xs_tiles[tt] = xpool.tile([128, 16, TT], f16, tag="xs", name="xs_sb")
            t0 = tt * TT
            for kc in range(kc_lo, kc_hi):
                nc.sync.dma_start(xs_tiles[tt][:, kc, :], xs[kc, :, t0 : t0 + TT])

        qk_tiles = {}

        def proj_qk(tt, fb):
            """Build projection emitters for q/k feature-block fb of tile tt.

            Returns (stream, finish): stream is a list of matmul thunks to
            interleave with other PSUM banks; finish emits the evacuations."""
            if tt >= NTT:
                return [], lambda: None
            if tt not in qk_tiles:
                qk_tiles[tt] = (
                    qkpool.tile([128, 4 * TT], f16, tag="qT", name="qT_sb"),
                    qkpool.tile([128, 4 * TT], f16, tag="kT", name="kT_sb"),
                )
            qT, kT = qk_tiles[tt]
            xt = xs_tiles[tt]
            pq = pspool.tile([128, 512], f32, tag="work", name="pq")
            pk = pspool.tile([128, 512], f32, tag="work", name="pk")

            def mm_pq(kc, pq=pq, xt=xt, fb=fb):
                nc.tensor.matmul(
                    pq,
                    wq_sb[:, kc, fb * 128 : (fb + 1) * 128],
                    xt[:, kc, :],
                    start=(kc == 0),
                    stop=(kc == 7),
                )

            def mm_pk(i, pk=pk, xt=xt, fb=fb):
                wc, xc = SPLIT3[i]
                nc.tensor.matmul(
                    pk,
                    wk_sb[:, wc, fb * 128 : (fb + 1) * 128],
                    xt[:, xc, :],
                    start=(i == 0),
                    stop=(i == len(SPLIT3) - 1),
                )

            # order pk/pq so adjacent emissions hit alternating PSUM banks
            # once merged with the pv stream: [pk*16, (pk,pq)*8]
            stream = [lambda i=i: mm_pk(i) for i in range(16)]
            for i in range(8):
                stream.append(lambda i=i: mm_pk(16 + i))
                stream.append(lambda i=i: mm_pq(i))

            def finish(qT=qT, kT=kT, pq=pq, pk=pk, fb=fb):
                nc.scalar.sign(kT[:, fb * TT : (fb + 1) * TT], pk)
                nc.scalar.copy(qT[:, fb * TT : (fb + 1) * TT], pq)

            return stream, finish

        # prologue: x(0) + full projection of tile 0 (fb pairs interleaved
        # so adjacent matmuls target different PSUM banks)
        dma_x(0, 0, 16)
        wv_sb = const.tile([128, 16, 512], f16)
        for kc in range(16):
            nc.sync.dma_start(wv_sb[:, kc, :], wv[kc])
        for fbp in range(2):
            s_a, f_a = proj_qk(0, 2 * fbp)
            s_b, f_b = proj_qk(0, 2 * fbp + 1)
            for a, b in zip(s_a, s_b):
                a()
                b()
            f_a()
            f_b()
        dma_x(1, 0, 16)

        # remaining constants (deferred so they queue behind the hot DMAs)
        wo_sb = const.tile([128, 4, 1024], f16)
        nc.sync.dma_start(wo_sb, wo.rearrange("jc p i -> p jc i"))
        inv_sb = const.tile([128, T], f32)
        nc.sync.dma_start(inv_sb, invtot)
        mask_sb = const.tile([128, 256], f32)
        nc.sync.dma_start(mask_sb, mask2)
        id_sb = const.tile([128, 128], f16)
        nc.sync.dma_start(id_sb, ident)

        # running state S[kf, df] fp16 (exact ints), 2 heads stacked per tile
        s_cur = spool.tile([128, 256], f16, tag="S", name="s_init")
        nc.vector.memset(s_cur, 0.0)

        for tt in range(NTT):
            qT, kT = qk_tiles[tt]
            xt = xs_tiles[tt]
            for ci in range(4):
                n = tt * 4 + ci
                c0 = ci * CH

                # interleave next tile's x DMA + projection to keep PE dense
                # prefetch x two tiles ahead
                if ci == 0:
                    dma_x(tt + 2, 0, 16)

                # --- scoresT[j, i] + k transposes, banks alternating.
                # Heads sharing a PSUM tile use the same PE row-group
                # (concurrent different-row matmuls on one bank collide
                # fatally); emission alternates banks to hide PSUM drains. ---
                scs = [
                    pspool.tile([128, 256], f32, tag="work", name=f"sc{p}")
                    for p in range(4)
                ]
                ktp = pspool.tile([128, 512], f16, tag="work", name="ktp")

                def mm_score(h):
                    p = (h % 2) + 2 * (h // 4)
                    idx = (h // 2) % 2
                    bp = (h % 2) * 64
                    hp = h // 2
                    sl = slice(hp * TT + c0, hp * TT + c0 + CH)
                    nc.tensor.matmul(
                        scs[p][:, idx * 128 : (idx + 1) * 128],
                        kT[bp : bp + 64, sl],
                        qT[bp : bp + 64, sl],
                        start=True,
                        stop=True,
                    )

                for h in range(4):
                    mm_score(h)
                for hp in range(4):
                    mm_score(4 + hp)
                    sl = slice(hp * TT + c0, hp * TT + c0 + CH)
                    nc.tensor.transpose(
                        ktp[:, hp * 128 : (hp + 1) * 128], kT[:, sl], id_sb
                    )
                sts = []
                for p in range(4):
                    st = stpool.tile([128, 256], f16, tag="st")
                    nc.vector.tensor_mul(st, scs[p], mask_sb)
                    sts.append(st)
                ktok = tokpool.tile([128, 512], f16, tag="ktok")
                nc.vector.tensor_copy(ktok, ktp)

                # --- merged projection stream: v (this chunk) + q/k of the
                # next tile, alternating PSUM banks to hide drain bubbles ---
                pv = pspool.tile([128, 512], f32, tag="work", name="pv")

                def mm_pv(i):
                    wc, xc = SPLIT3[i]
                    nc.tensor.matmul(
                        pv,
                        xt[:, xc, c0 : c0 + CH],
                        wv_sb[:, wc, :],
                        start=(i == 0),
                        stop=(i == len(SPLIT3) - 1),
                    )

                qk_stream, qk_finish = proj_qk(tt + 1, ci)
                for i in range(max(24, len(qk_stream))):
                    if i < 24:
                        mm_pv(i)
                    if i < len(qk_stream):
                        qk_stream[i]()
                qk_finish()
                vtok = tokpool.tile([128, 512], f16, tag="vtok")
                nc.scalar.sign(vtok, pv)

                # --- ckv[kf, df] + outT = intraT + crossT, interleaved ---
                ckv = pspool.tile([128, 256], f32, tag="work", name="ckv")
                ot = pspool.tile([128, 512], f32, tag="work", name="ot")
                for h in range(8):
                    bp = (h % 2) * 64
                    hp = h // 2
                    sp_ = (h % 2) + 2 * (h // 4)
                    si_ = (h // 2) % 2
                    nc.tensor.matmul(
                        ot[bp : bp + 64, hp * 128 : hp * 128 + 128],
                        vtok[:, h * 64 : h * 64 + 64],
                        sts[sp_][:, si_ * 128 : si_ * 128 + 128],
                        start=True,
                        stop=(n == 0),
                        tile_position=(0, bp),
                    )
                    if n > 0:
                        nc.tensor.matmul(
                            ot[bp : bp + 64, hp * 128 : hp * 128 + 128],
                            s_cur[bp : bp + 64, hp * 64 : hp * 64 + 64],
                            qT[bp : bp + 64, hp * TT + c0 : hp * TT + c0 + CH],
                            start=False,
                            stop=True,
                            tile_position=(bp, bp),
                        )
                    nc.tensor.matmul(
                        ckv[bp : bp + 64, (h // 2) * 64 : (h // 2) * 64 + 64],
                        ktok[:, h * 64 : h * 64 + 64],
                        vtok[:, h * 64 : h * 64 + 64],
                        start=True,
                        stop=True,
                        tile_position=(0, bp),
                    )

                # --- state update S += ckv (exact ints in fp16) ---
                s_new = spool.tile([128, 256], f16, tag="S", name="s_new")
                nc.vector.tensor_add(s_new, s_cur, ckv)
                s_cur = s_new

                # --- scale by 1/(8*(t+1)), cast to fp16 for o_proj ---
                osb = opool.tile([128, 512], f16, tag="osb")
                for hp in range(4):
                    nc.vector.tensor_mul(
                        osb[:, hp * 128 : (hp + 1) * 128],
                        ot[:, hp * 128 : (hp + 1) * 128],
                        inv_sb[:, n * CH : n * CH + CH],
                    )

                # --- o_proj (fp16): y[t, i] partial over this core's j-feats ---
                ysb = ypool.tile([128, 1024], f32, tag="ysb")
                yps = [
                    pspool.tile([128, 512], f32, tag="work", name=f"yp{icol}")
                    for icol in range(2)
                ]
                for hp in range(4):
                    for icol in range(2):
                        nc.tensor.matmul(
                            yps[icol],
                            osb[:, hp * 128 : (hp + 1) * 128],
                            wo_sb[:, hp, icol * 512 : (icol + 1) * 512],
                            start=(hp == 0),
                            stop=(hp == 3),
                        )
                for icol in range(2):
                    nc.scalar.copy(ysb[:, icol * 512 : (icol + 1) * 512], yps[icol])
                nc.sync.dma_start(ypart[n * CH : (n + 1) * CH, :], ysb)

        # --- final matrix (exact integer sums) ---
        fsb = opool.tile([128, 256], f32, tag="fsb")
        nc.vector.tensor_copy(fsb, s_cur)
        for h in range(8):
            nc.sync.dma_start(
                fmat[h],
                fsb[(h % 2) * 64 : (h % 2) * 64 + 64, (h // 2) * 64 : (h // 2) * 64 + 64],
            )

    nc.compile()
    return nc


def _split16(a):
    """fp16 double-double split along axis 0-stacking: returns (hi, lo)."""
    hi = a.astype(np.float16)
    lo = (a - hi.astype(np.float32)).astype(np.float16)
    return hi, lo


def _host_inputs(x, W_qkv, W_o):
    f32 = np.float32
    tvec = np.arange(1, T + 1, dtype=np.float64)
    inv = (1.0 / (8.0 * tvec)).astype(f32)
    invtot = np.ascontiguousarray(np.broadcast_to(inv[None, :], (128, T)))
    jj = np.arange(128)
    maskT = (jj[:, None] <= jj[None, :]).astype(f32)
    mask2 = np.ascontiguousarray(np.tile(maskT, (1, 2)))
    ident = np.eye(128, dtype=np.float16)

    Wq3 = np.asarray(W_qkv, dtype=f32).reshape(3, 16, DH, D)
    in_maps = []
    for core in range(8):
        b, g = core // 2, core % 2
        xT = np.asarray(x[b], dtype=f32).T  # [D, T]
        x1, x2 = _split16(xT)
        xs = np.ascontiguousarray(
            np.concatenate([x1, x2], axis=0).reshape(16, 128, T)
        )
        # weight column blocks for this head group, [D, 512] each
        wqc = Wq3[0, g * 8 : (g + 1) * 8].reshape(512, D).T
        wkc = Wq3[1, g * 8 : (g + 1) * 8].reshape(512, D).T
        wvc = Wq3[2, g * 8 : (g + 1) * 8].reshape(512, D).T
        wq = np.ascontiguousarray(wqc.astype(np.float16).reshape(8, 128, 512))
        wk1, wk2 = _split16(wkc)
        wk = np.ascontiguousarray(np.concatenate([wk1, wk2], axis=0).reshape(16, 128, 512))
        wv1, wv2 = _split16(wvc)
        wv = np.ascontiguousarray(np.concatenate([wv1, wv2], axis=0).reshape(16, 128, 512))
        wo = np.ascontiguousarray(
            np.asarray(W_o, dtype=f32)[:, g * 512 : (g + 1) * 512].T.astype(np.float16).reshape(4, 128, D)
        )
        in_maps.append(
            {
                "xs": xs,
                "wq": wq,
                "wk": wk,
                "wv": wv,
                "wo": wo,
                "invtot": invtot,
                "mask2": mask2,
                "ident": ident,
            }
        )
    return in_maps


def kernel(x, W_qkv, W_o, trace=False):
    from concourse import bass_utils

    nc = _build()
    in_maps = _host_inputs(x, W_qkv, W_o)
    res = bass_utils.run_bass_kernel_spmd(
        nc, in_maps, core_ids=list(range(8)), trace=trace
    )
    results = res.results

    f32 = np.float32
    y = np.empty((4, T, D), dtype=f32)
    fm = np.empty((4, 16, DH, DH), dtype=f32)
    for core in range(8):
        b, g = core // 2, core % 2
        if g == 0:
            y[b] = results[core]["ypart"]
        else:
            y[b] += results[core]["ypart"]
        fm[b, g * 8 : (g + 1) * 8] = results[core]["fmat"]
    fc = np.full((4, 16, 1, 1), float(T), dtype=f32)
    if trace:
        kernel._last_result = res
    return (y, fm, fc)


# revision 15
# speedup vs baseline: 1.0878x; 1.0256x over previous
"""Binary associative memory (causal linear attention with binarized k/v).

Self-contained Trainium2 Bass kernel.

Math: the reference's chunked prefix recurrence telescopes to exact causal
linear attention:
    out[t] = (1/(8*(t+1))) * sum_{s<=t} (q[t].k[s]) v[s],   k,v = sign(qkv)
    y      = out @ W_o.T   (summed over head features)
    final_matrix[b,h] = sum_t k[t] (x) v[t]   (exact integers)
so we are free to re-chunk at 128 tokens (partition width).

Sharding: 8 cores = 4 batches x 2 head-groups (8 heads each).

Precision: k/v binarize must match the fp32 reference's signs, so the k/v
projections use an fp16 double-double trick: x ~= x1 + x2 and W ~= w1 + w2
(fp16 splits, prepared on host) stacked along the contraction dim, so one
K=2048 accumulation computes (x1+x2)@(w1+w2) with ~1e-7 relative error at
full PE speed. q (continuous) uses a single fp16 term; o_proj runs fp16.

Layouts per core:
  - q,k projected feature-on-partition ([feat, tok]) for scoresT/crossT
  - v projected token-on-partition directly ([tok, vfeat]) for intra/ckv
  - k transposed per chunk on the PE (with identity) for ckv
  - state S[kf, df] fp16 (exact small integers), 2 heads per 128 partitions
"""

import functools

import numpy as np

T = 4096
D = 1024
HLOC = 8  # heads per core
DH = 64
CH = 128  # chunk size
NCH = T // CH  # 32
TT = 512  # projection token tile
NTT = T // TT  # 8

PAIRS = [(0, 2), (1, 3), (4, 6), (5, 7)]  # scores PSUM-tile pairs (same row-group)
# 3-term fp16 split of x@W: x1w1 + x2w1 + x1w2 (chunk indices into [hi(8), lo(8)])
SPLIT3 = [(wc, xc) for wc, xc in
          [(c, c) for c in range(8)] + [(c, c + 8) for c in range(8)] + [(c + 8, c) for c in range(8)]]


@functools.lru_cache(maxsize=1)
def _build():
    from contextlib import ExitStack

    import concourse.bacc as bacc
    import concourse.mybir as mybir
    import concourse.tile as tile

    f32 = mybir.dt.float32
    f16 = mybir.dt.float16

    nc = bacc.Bacc("TRN2", target_bir_lowering=False, debug=False, num_devices=8)

    # x split [x1; x2], chunk-major: [16 kc][128][T]
    xs = nc.dram_tensor("xs", [16, 128, T], f16, kind="ExternalInput").ap()
    wq = nc.dram_tensor("wq", [8, 128, 512], f16, kind="ExternalInput").ap()
    wk = nc.dram_tensor("wk", [16, 128, 512], f16, kind="ExternalInput").ap()
    wv = nc.dram_tensor("wv", [16, 128, 512], f16, kind="ExternalInput").ap()
    wo = nc.dram_tensor("wo", [4, 128, 1024], f16, kind="ExternalInput").ap()
    invtot = nc.dram_tensor("invtot", [128, T], f32, kind="ExternalInput").ap()
    mask2 = nc.dram_tensor("mask2", [128, 256], f32, kind="ExternalInput").ap()
    ident = nc.dram_tensor("ident", [128, 128], f16, kind="ExternalInput").ap()
    ypart = nc.dram_tensor("ypart", [T, D], f32, kind="ExternalOutput").ap()
    fmat = nc.dram_tensor("fmat", [HLOC, DH, DH], f32, kind="ExternalOutput").ap()

    with tile.TileContext(nc) as tc, ExitStack() as ctx:
        const = ctx.enter_context(tc.tile_pool(name="const", bufs=1))
        xpool = ctx.enter_context(tc.tile_pool(name="xp", bufs=3))
        qkpool = ctx.enter_context(tc.tile_pool(name="qk", bufs=2))
        tokpool = ctx.enter_context(tc.tile_pool(name="tok", bufs=3))
        stpool = ctx.enter_context(tc.tile_pool(name="st", bufs=8))
        opool = ctx.enter_context(tc.tile_pool(name="op", bufs=3))
        ypool = ctx.enter_context(tc.tile_pool(name="yp", bufs=3))
        spool = ctx.enter_context(tc.tile_pool(name="sp", bufs=2))
        pspool = ctx.enter_context(tc.tile_pool(name="ps", bufs=8, space="PSUM"))

        # weights needed by the first projections: emit DMAs first
        wq_sb = const.tile([128, 8, 512], f16)
        for kc in range(8):
            nc.sync.dma_start(wq_sb[:, kc, :], wq[kc])
        wk_sb = const.tile([128, 16, 512], f16)
        for kc in range(16):
            nc.sync.dma_start(wk_sb[:, kc, :], wk[kc])

        xs_tiles = {}

        def dma_x(tt, kc_lo, kc_hi):
            if tt >= NTT:
                return
            if tt not in xs_tiles:
                xs_tiles[tt] = xpool.tile([128, 16, TT], f16, tag="xs", name="xs_sb")
            t0 = tt * TT
            for kc in range(kc_lo, kc_hi):
                nc.sync.dma_start(xs_tiles[tt][:, kc, :], xs[kc, :, t0 : t0 + TT])

        qk_tiles = {}

        def proj_qk(tt, fb):
            """Build projection emitters for q/k feature-block fb of tile tt.

            Returns (stream, finish): stream is a list of matmul thunks to
            interleave with other PSUM banks; finish emits the evacuations."""
            if tt >= NTT:
                return [], lambda: None
            if tt not in qk_tiles:
                qk_tiles[tt] = (
                    qkpool.tile([128, 4 * TT], f16, tag="qT", name="qT_sb"),
                    qkpool.tile([128, 4 * TT], f16, tag="kT", name="kT_sb"),
                )
            qT, kT = qk_tiles[tt]
            xt = xs_tiles[tt]
            pq = pspool.tile([128, 512], f32, tag="work", name="pq")
            pk = pspool.tile([128, 512], f32, tag="work", name="pk")

            def mm_pq(kc, pq=pq, xt=xt, fb=fb):
                nc.tensor.matmul(
                    pq,
                    wq_sb[:, kc, fb * 128 : (fb + 1) * 128],
                    xt[:, kc, :],
                    start=(kc == 0),
                    stop=(kc == 7),
                )

            def mm_pk(i, pk=pk, xt=xt, fb=fb):
                wc, xc = SPLIT3[i]
                nc.tensor.matmul(
                    pk,
                    wk_sb[:, wc, fb * 128 : (fb + 1) * 128],
                    xt[:, xc, :],
                    start=(i == 0),
                    stop=(i == len(SPLIT3) - 1),
                )

            # order pk/pq so adjacent emissions hit alternating PSUM banks
            # once merged with the pv stream: [pk*16, (pk,pq)*8]
            stream = [lambda i=i: mm_pk(i) for i in range(16)]
            for i in range(8):
                stream.append(lambda i=i: mm_pk(16 + i))
                stream.append(lambda i=i: mm_pq(i))

            def finish(qT=qT, kT=kT, pq=pq, pk=pk, fb=fb):
                nc.scalar.sign(kT[:, fb * TT : (fb + 1) * TT], pk)
                nc.scalar.copy(qT[:, fb * TT : (fb + 1) * TT], pq)

            return stream, finish

        # prologue: x(0) + full projection of tile 0 (fb pairs interleaved
        # so adjacent matmuls target different PSUM banks)
        dma_x(0, 0, 16)
        wv_sb = const.tile([128, 16, 512], f16)
        for kc in range(16):
            nc.sync.dma_start(wv_sb[:, kc, :], wv[kc])
        for fbp in range(2):
            s_a, f_a = proj_qk(0, 2 * fbp)
            s_b, f_b = proj_qk(0, 2 * fbp + 1)
            for a, b in zip(s_a, s_b):
                a()
                b()
            f_a()
            f_b()
        dma_x(1, 0, 16)

        # remaining constants (deferred so they queue behind the hot DMAs)
        wo_sb = const.tile([128, 4, 1024], f16)
        nc.sync.dma_start(wo_sb, wo.rearrange("jc p i -> p jc i"))
        inv_sb = const.tile([128, T], f32)
        nc.sync.dma_start(inv_sb, invtot)
        mask_sb = const.tile([128, 256], f32)
        nc.sync.dma_start(mask_sb, mask2)
        id_sb = const.tile([128, 128], f16)
        nc.sync.dma_start(id_sb, ident)

        # running state S[kf, df] fp16 (exact ints), 2 heads stacked per tile
        s_cur = spool.tile([128, 256], f16, tag="S", name="s_init")
        nc.vector.memset(s_cur, 0.0)

        prev_osb = None

        def emit_oproj(m, osb_m):
            # y[t, i] (fp16), partial over this core's 512 j-feats
            ysb = ypool.tile([128, 1024], f32, tag="ysb")
            yps = [
                pspool.tile([128, 512], f32, tag="work", name=f"yp{icol}")
                for icol in range(2)
            ]
            for hp in range(4):
                for icol in range(2):
                    nc.tensor.matmul(
                        yps[icol],
                        osb_m[:, hp * 128 : (hp + 1) * 128],
                        wo_sb[:, hp, icol * 512 : (icol + 1) * 512],
                        start=(hp == 0),
                        stop=(hp == 3),
                    )
            for icol in range(2):
                nc.scalar.copy(ysb[:, icol * 512 : (icol + 1) * 512], yps[icol])
            nc.sync.dma_start(ypart[m * CH : (m + 1) * CH, :], ysb)

        for tt in range(NTT):
            qT, kT = qk_tiles[tt]
            xt = xs_tiles[tt]
            for ci in range(4):
                n = tt * 4 + ci
                c0 = ci * CH

                # interleave next tile's x DMA + projection to keep PE dense
                # prefetch x two tiles ahead, spread across chunks
                dma_x(tt + 2, ci * 4, ci * 4 + 4)

                # --- scoresT[j, i] + k transposes, banks alternating.
                # Heads sharing a PSUM tile use the same PE row-group
                # (concurrent different-row matmuls on one bank collide
                # fatally); emission alternates banks to hide PSUM drains. ---
                scs = [
                    pspool.tile([128, 256], f32, tag="work", name=f"sc{p}")
                    for p in range(4)
                ]
                ktp = pspool.tile([128, 512], f16, tag="work", name="ktp")

                def mm_score(h):
                    p = (h % 2) + 2 * (h // 4)
                    idx = (h // 2) % 2
                    bp = (h % 2) * 64
                    hp = h // 2
                    sl = slice(hp * TT + c0, hp * TT + c0 + CH)
                    nc.tensor.matmul(
                        scs[p][:, idx * 128 : (idx + 1) * 128],
                        kT[bp : bp + 64, sl],
                        qT[bp : bp + 64, sl],
                        start=True,
                        stop=True,
                    )

                for h in range(4):
                    mm_score(h)
                for hp in range(4):
                    mm_score(4 + hp)
                    sl = slice(hp * TT + c0, hp * TT + c0 + CH)
                    nc.tensor.transpose(
                        ktp[:, hp * 128 : (hp + 1) * 128], kT[:, sl], id_sb
                    )
                sts = []
                for p in range(4):
                    st = stpool.tile([128, 256], f16, tag="st")
                    nc.vector.tensor_mul(st, scs[p], mask_sb)
                    sts.append(st)
                ktok = tokpool.tile([128, 512], f16, tag="ktok")
                nc.vector.tensor_copy(ktok, ktp)

                # --- o_proj of the PREVIOUS chunk (osb ready; keeps the PE
                # array busy with N=512 work between the smalls) ---
                if prev_osb is not None:
                    emit_oproj(n - 1, prev_osb)
                prev_osb = None

                # --- merged projection stream: v (this chunk) + q/k of the
                # next tile, alternating PSUM banks to hide drain bubbles ---
                pv = pspool.tile([128, 512], f32, tag="work", name="pv")

                def mm_pv(i):
                    wc, xc = SPLIT3[i]
                    nc.tensor.matmul(
                        pv,
                        xt[:, xc, c0 : c0 + CH],
                        wv_sb[:, wc, :],
                        start=(i == 0),
                        stop=(i == len(SPLIT3) - 1),
                    )

                qk_stream, qk_finish = proj_qk(tt + 1, ci)
                for i in range(max(24, len(qk_stream))):
                    if i < 24:
                        mm_pv(i)
                    if i < len(qk_stream):
                        qk_stream[i]()
                qk_finish()
                vtok = tokpool.tile([128, 512], f16, tag="vtok")
                nc.scalar.sign(vtok, pv)

                # --- ckv[kf, df] + outT = intraT + crossT, interleaved ---
                ckv = pspool.tile([128, 256], f32, tag="work", name="ckv")
                ot = pspool.tile([128, 512], f32, tag="work", name="ot")
                for h in range(8):
                    bp = (h % 2) * 64
                    hp = h // 2
                    sp_ = (h % 2) + 2 * (h // 4)
                    si_ = (h // 2) % 2
                    nc.tensor.matmul(
                        ot[bp : bp + 64, hp * 128 : hp * 128 + 128],
                        vtok[:, h * 64 : h * 64 + 64],
                        sts[sp_][:, si_ * 128 : si_ * 128 + 128],
                        start=True,
                        stop=(n == 0),
                        tile_position=(0, bp),
                    )
                    if n > 0:
                        nc.tensor.matmul(
                            ot[bp : bp + 64, hp * 128 : hp * 128 + 128],
                            s_cur[bp : bp + 64, hp * 64 : hp * 64 + 64],
                            qT[bp : bp + 64, hp * TT + c0 : hp * TT + c0 + CH],
                            start=False,
                            stop=True,
                            tile_position=(bp, bp),
                        )
                    nc.tensor.matmul(
                        ckv[bp : bp + 64, (h // 2) * 64 : (h // 2) * 64 + 64],
                        ktok[:, h * 64 : h * 64 + 64],
                        vtok[:, h * 64 : h * 64 + 64],
                        start=True,
                        stop=True,
                        tile_position=(0, bp),
                    )

                # --- state update S += ckv (exact ints in fp16) ---
                s_new = spool.tile([128, 256], f16, tag="S", name="s_new")
                nc.vector.tensor_add(s_new, s_cur, ckv)
                s_cur = s_new

                # --- scale by 1/(8*(t+1)), cast to fp16 for o_proj ---
                osb = opool.tile([128, 512], f16, tag="osb")
                for hp in range(4):
                    nc.vector.tensor_mul(
                        osb[:, hp * 128 : (hp + 1) * 128],
                        ot[:, hp * 128 : (hp + 1) * 128],
                        inv_sb[:, n * CH : n * CH + CH],
                    )

                prev_osb = osb

        emit_oproj(NCH - 1, prev_osb)

        # --- final matrix (exact integer sums) ---
        fsb = opool.tile([128, 256], f32, tag="fsb")
        nc.vector.tensor_copy(fsb, s_cur)
        for h in range(8):
            nc.sync.dma_start(
                fmat[h],
                fsb[(h % 2) * 64 : (h % 2) * 64 + 64, (h // 2) * 64 : (h // 2) * 64 + 64],
            )

    nc.compile()
    return nc


def _split16(a):
    """fp16 double-double split along axis 0-stacking: returns (hi, lo)."""
    hi = a.astype(np.float16)
    lo = (a - hi.astype(np.float32)).astype(np.float16)
    return hi, lo


def _host_inputs(x, W_qkv, W_o):
    f32 = np.float32
    tvec = np.arange(1, T + 1, dtype=np.float64)
    inv = (1.0 / (8.0 * tvec)).astype(f32)
    invtot = np.ascontiguousarray(np.broadcast_to(inv[None, :], (128, T)))
    jj = np.arange(128)
    maskT = (jj[:, None] <= jj[None, :]).astype(f32)
    mask2 = np.ascontiguousarray(np.tile(maskT, (1, 2)))
    ident = np.eye(128, dtype=np.float16)

    Wq3 = np.asarray(W_qkv, dtype=f32).reshape(3, 16, DH, D)
    in_maps = []
    for core in range(8):
        b, g = core // 2, core % 2
        xT = np.asarray(x[b], dtype=f32).T  # [D, T]
        x1, x2 = _split16(xT)
        xs = np.ascontiguousarray(
            np.concatenate([x1, x2], axis=0).reshape(16, 128, T)
        )
        # weight column blocks for this head group, [D, 512] each
        wqc = Wq3[0, g * 8 : (g + 1) * 8].reshape(512, D).T
        wkc = Wq3[1, g * 8 : (g + 1) * 8].reshape(512, D).T
        wvc = Wq3[2, g * 8 : (g + 1) * 8].reshape(512, D).T
        wq = np.ascontiguousarray(wqc.astype(np.float16).reshape(8, 128, 512))
        wk1, wk2 = _split16(wkc)
        wk = np.ascontiguousarray(np.concatenate([wk1, wk2], axis=0).reshape(16, 128, 512))
        wv1, wv2 = _split16(wvc)
        wv = np.ascontiguousarray(np.concatenate([wv1, wv2], axis=0).reshape(16, 128, 512))
        wo = np.ascontiguousarray(
            np.asarray(W_o, dtype=f32)[:, g * 512 : (g + 1) * 512].T.astype(np.float16).reshape(4, 128, D)
        )
        in_maps.append(
            {
                "xs": xs,
                "wq": wq,
                "wk": wk,
                "wv": wv,
                "wo": wo,
                "invtot": invtot,
                "mask2": mask2,
                "ident": ident,
            }
        )
    return in_maps


def kernel(x, W_qkv, W_o, trace=False):
    from concourse import bass_utils

    nc = _build()
    in_maps = _host_inputs(x, W_qkv, W_o)
    res = bass_utils.run_bass_kernel_spmd(
        nc, in_maps, core_ids=list(range(8)), trace=trace
    )
    results = res.results

    f32 = np.float32
    y = np.empty((4, T, D), dtype=f32)
    fm = np.empty((4, 16, DH, DH), dtype=f32)
    for core in range(8):
        b, g = core // 2, core % 2
        if g == 0:
            y[b] = results[core]["ypart"]
        else:
            y[b] += results[core]["ypart"]
        fm[b, g * 8 : (g + 1) * 8] = results[core]["fmat"]
    fc = np.full((4, 16, 1, 1), float(T), dtype=f32)
    if trace:
        kernel._last_result = res
    return (y, fm, fc)
